# revision 16
# baseline (speedup 1.0000x reference)
"""Trainium2 Bass kernel for nn_AME2Encoder (dense_mlp, 8-core data parallel).

Strategy:
  - Pure data parallel: B=2048 sharded 256/core; each core processes its
    samples as 128 "pairs" (2 samples packed on the 128 SBUF partitions),
    with most elementwise work done on 2-pair-wide [128, 1008] tiles to
    amortize per-instruction overheads.
  - Feature-major bf16 activations ([feat, token] tiles, token tile = 504
    = one sample's full 14x36 grid). No transposes needed anywhere.
  - 64-feature layers are packed 2-samples-per-matmul with block-diagonal
    weights (M=128).
  - conv1 (3x3) is a single K=54 matmul per pair over a host-prepared
    im2col layout (input layout prep; all FLOPs stay on device).
  - ELU in 3 passes via the "+1 fold": every ELU site computes
    elu(x)+1 = min(exp(x+b),1) + relu(x+b); the -1 is folded into the next
    layer's bias on the host (scores are softmax-shift-invariant for K,
    and V/global-max shifts fold into constants).
      pass1: ACT Exp(psum + bias) -> e (bf16)
      pass2: ACT Relu(psum + bias) or DVE tensor_scalar -> r (bf16)
      pass3: scalar_tensor_tensor (e min 1) add r -> out (one fused pass)
  - Attention: block-diag 0.5*Q masks -> one scores matmul per pair; ACT
    exp with free sum accumulation; fused V-weighting + context reduction
    in one scalar_tensor_tensor with accum_out.
  - K/V biases: bk drops out of softmax; bv folded into the context
    normalization. So K|V evacuation is a single wide pure-copy pass.
"""

import os
from contextlib import ExitStack

import numpy as np
import ml_dtypes

import concourse.bass as bass
import concourse.mybir as mybir
import concourse.tile as tile
from concourse.bass_utils import run_bass_kernel_spmd
from concourse.vector_clock import ScopedClock


# --- workaround: this walrus rejects the tail Drain carrying >1 sem waits ---
def _patched_dab(self, tick_clock, wait_clock):
    nc = self.nc
    probe = nc.sync.drain()
    wait_clock.add_sem_waits(probe.ins, ScopedClock({None: tick_clock.global_clock}))
    si = probe.ins.sync_info
    waits = list(si.on_wait) if si is not None else []
    if si is not None and len(waits) > 1:
        si.on_wait = waits[:1]
        for w in waits[1:]:
            n2 = nc.sync.drain()
            n2.ins.sync_info = mybir.SyncInfo(on_wait=[w], on_update=[])
    nc.all_engine_barrier()
    assert self.sems is not None
    popped = nc._tile_sem_poison_stack.pop()
    assert popped is self._sem_poison
    nc.clear_and_free_semaphores(list(self.sems.allocated().values()))
    nc.all_engine_barrier()


tile.TileContext._drain_and_barrier = _patched_dab


def _split_multiwait(nc, max_waits=1):
    """This walrus build cannot encode >1 sem-wait on one instruction for some
    structs; hoist excess waits onto EventSemaphore carriers inserted before."""
    ctr = [0]
    for fn in nc.m.functions:
        for blk in fn.blocks:
            insts = list(blk.instructions)
            new = []
            changed = False
            for inst in insts:
                si = inst.sync_info
                waits = list(si.on_wait) if si is not None and si.on_wait else []
                if len(waits) > max_waits:
                    changed = True
                    for w in waits[max_waits:]:
                        ctr[0] += 1
                        new.append(mybir.InstEventSemaphore(
                            name=f"zz_mw_{ctr[0]}", engine=inst.engine,
                            ins=[], outs=[],
                            sync_info=mybir.SyncInfo(on_wait=[w], on_update=[]),
                        ))
                    inst.sync_info = mybir.SyncInfo(
                        on_wait=waits[:max_waits],
                        on_update=list(si.on_update) if si.on_update else [],
                    )
                new.append(inst)
            if changed:
                blk.instructions = new


# ----- problem constants (hardcoded per spec) -----
B, C_IN, H, W = 2048, 3, 14, 36
D_LOCAL, D_POS, D_GLOBAL, D_PROP, NH = 64, 64, 128, 128, 16
HD = D_LOCAL // NH
N_CORES = 8
B_LOC = B // N_CORES      # 256
NPAIR = B_LOC // 2        # 128
NQUAD = NPAIR // 2        # 64 wide iterations (2 pairs each)
L = H * W                 # 504
LP = 512                  # PSUM-bank-aligned half stride
LW = LP + L               # 1016: wide tile width (second half at [LP, LP+L))

BF = mybir.dt.float16
F32 = mybir.dt.float32
bf16 = np.float16
AX = mybir.AluOpType
AF = mybir.ActivationFunctionType

# engine assignment for the flexible passes ("act" or "dve"); stt passes may
# also go to "gp" (gpsimd).
ASSIGN = {
    "r_conv1": "act", "r_conv2": "act", "r_fuse": "act", "r_g1": "dve",
    "kvevac": "act",
    "stt_conv1": "dve", "stt_conv2": "dve", "stt_fuse": "dve", "stt_g1": "dve",
}


def _np_elu(x):
    return np.where(x > 0, x, np.expm1(np.minimum(x, 0.0)))


# ----------------------------------------------------------------------------
# Host-side constant packing (weight folding / layout prep)
# ----------------------------------------------------------------------------

def _block_diag2(w):
    k, m = w.shape
    out = np.zeros((2 * k, 2 * m), np.float32)
    out[:k, :m] = w
    out[k:, m:] = w
    return out


def _dup_col(b):
    return np.concatenate([b, b]).astype(np.float32)[:, None]


def host_prep_shared(inp):
    """Weight-derived dram parameters. All ELU sites produce elu(x)+1; the -1
    is folded into each consumer's bias here (colsum of the consumer weight)."""
    c = {}
    w1p = inp["conv1_w"].transpose(2, 3, 1, 0).reshape(27, 64)  # k=(3dy+dx)*3+c
    c["w1bd"] = _block_diag2(w1p).astype(bf16)            # [54,128]
    c["b1d"] = _dup_col(inp["conv1_b"])                   # [128,1] f32

    w2 = inp["conv2_w"][:, :, 0, 0].T                     # [in,out]
    c["w2bd"] = _block_diag2(w2).astype(bf16)             # [128,128]
    c["b2d"] = _dup_col(inp["conv2_b"])

    fl = inp["fuse_w"][:D_LOCAL]                          # [64,64]
    fp = inp["fuse_w"][D_LOCAL:]                          # [64,64]
    c["wflbd"] = _block_diag2(fl).astype(bf16)            # [128,128]
    c["wfp2"] = np.concatenate([fp, fp], axis=1).astype(bf16)  # [64,128]
    c["bfd"] = _dup_col(inp["fuse_b"])

    ys = np.linspace(-1.0, 1.0, H, dtype=np.float32)
    xs = np.linspace(-1.0, 1.0, W, dtype=np.float32)
    gy, gx = np.meshgrid(ys, xs, indexing="ij")
    coords = np.stack([gx, gy], axis=-1).reshape(L, 2)
    pe = _np_elu(coords @ inp["pe_w1"] + inp["pe_b1"]) @ inp["pe_w2"] + inp["pe_b2"]
    c["pec"] = np.ascontiguousarray(pe.T).astype(bf16)    # [64,504] exact

    g1 = inp["g_w1"]                                      # [64,128]
    c["g1w2"] = np.vstack([g1, g1]).astype(bf16)          # [128,128] dup rows
    c["bg1d"] = inp["g_b1"].astype(np.float32)[:, None]

    g2 = inp["g_w2"]
    c["g2w"] = g2.astype(bf16)                            # [128,128]
    c["bg2"] = inp["g_b2"].astype(np.float32)[:, None]    # applied post-gmax

    c["wvbd"] = _block_diag2(inp["wv"]).astype(bf16)      # [128,128]
    c["wkbd"] = _block_diag2(inp["wk"]).astype(bf16)      # bk drops in softmax
    c["cvd"] = _dup_col(inp["bv"])  # folded into ctx normalization

    sm = np.zeros((64, 64), np.float32)
    for k in range(64):
        sm[k, (k // HD) * HD:(k // HD + 1) * HD] = 1.0 / np.sqrt(HD)
    c["csmbd"] = _block_diag2(sm).astype(bf16)            # [128,128]

    c["qpwg"] = inp["qp_w"][:D_GLOBAL].astype(bf16)       # [128,64]
    c["qpwp"] = inp["qp_w"][D_GLOBAL:].astype(bf16)       # [128,64]
    c["qpb"] = inp["qp_b"].astype(np.float32)[:, None]    # [64,1]

    c["wq"] = inp["wq"].astype(bf16)
    c["bq2"] = _dup_col(inp["bq"])                        # [128,1]

    c["wobd"] = _block_diag2(inp["wo"]).astype(bf16)      # [128,128]
    c["bod"] = _dup_col(inp["bo"])
    return c


def host_prep_percore(inp):
    """Per-core data params: im2col input layout + transposed prop_emb."""
    mf = inp["map_feat"].astype(np.float32)
    mp = np.zeros((B, 3, H + 2, W + 2), np.float32)
    mp[:, :, 1:H + 1, 1:W + 1] = mf
    from numpy.lib.stride_tricks import sliding_window_view
    sw = sliding_window_view(mp, (3, 3), axis=(2, 3))     # [B,3,14,36,3,3]
    ic = sw.transpose(0, 4, 5, 1, 2, 3).reshape(B, 27, L)  # k=(3dy+dx)*3+c
    ic = np.ascontiguousarray(ic).reshape(B // 2, 54, L).astype(bf16)
    prop = inp["prop_emb"].astype(np.float32)
    cores = []
    for ci in range(N_CORES):
        sl = slice(ci * B_LOC, (ci + 1) * B_LOC)
        cores.append({
            "ic": np.ascontiguousarray(ic[ci * NPAIR:(ci + 1) * NPAIR]),
            "propT": np.ascontiguousarray(prop[sl].T).astype(bf16),  # [128,256]
        })
    return cores


# ----------------------------------------------------------------------------
# Bass graph
# ----------------------------------------------------------------------------

def build_nc(shared):
    nc = bass.Bass()

    P = {}
    P["ic"] = nc.declare_dram_parameter("ic", [NPAIR, 54, L], BF, isOutput=False)
    P["propT"] = nc.declare_dram_parameter("propT", [D_PROP, B_LOC], BF,
                                           isOutput=False)
    for name, arr in shared.items():
        dt = BF if arr.dtype == bf16 else F32
        P[name] = nc.declare_dram_parameter(name, list(arr.shape), dt,
                                            isOutput=False)
    out_h = nc.declare_dram_parameter("out", [B_LOC, D_LOCAL + D_GLOBAL], F32,
                                      isOutput=True)

    def dram_ap(h, offset, dims):
        base = h[:]
        return bass.AP(tensor=base.tensor, offset=offset,
                       ap=[list(d) for d in dims])

    with tile.TileContext(nc) as tc, ExitStack() as ctx:
        singles = ctx.enter_context(tc.tile_pool(name="singles", bufs=1))
        pA_in = ctx.enter_context(tc.tile_pool(name="pA_in", bufs=2))
        pA_sb = ctx.enter_context(tc.tile_pool(name="pA_sb", bufs=3))
        pC_sb = ctx.enter_context(tc.tile_pool(name="pC_sb", bufs=2))
        psA = ctx.enter_context(tc.tile_pool(name="psA", bufs=2, space="PSUM"))
        psG = ctx.enter_context(tc.tile_pool(name="psG", bufs=2, space="PSUM"))

        # ---- constants ----
        cs = {}
        for name, arr in shared.items():
            dt = BF if arr.dtype == bf16 else F32
            t = singles.tile(list(arr.shape), dt, tag=f"c_{name}",
                             name=f"c_{name}")
            nc.sync.dma_start(out=t[:], in_=P[name][:])
            cs[name] = t
        cprop = singles.tile([D_PROP, B_LOC], BF, tag="c_prop", name="c_prop")
        nc.sync.dma_start(out=cprop[:], in_=P["propT"][:])

        # persistent state
        gf_all = singles.tile([D_GLOBAL, B_LOC], F32, tag="gf_all", name="gf_all")
        gf_bf = singles.tile([D_GLOBAL, B_LOC], BF, tag="gf_bf", name="gf_bf")
        ctx_all = singles.tile([128, NPAIR], BF, tag="ctx_all", name="ctx_all")
        Q2 = singles.tile([128, NPAIR], F32, tag="Q2", name="Q2")
        pw_tiles = [singles.tile([128, LW], BF, tag=f"pw{q}", name=f"pw{q}")
                    for q in range(NQUAD)]

        ENG = {"act": nc.scalar, "dve": nc.vector, "gp": nc.gpsimd}

        def elu1(pool, dst_ap, src_ap, bias_tile, site, nparts=128):
            """dst = elu(src + b) = min(exp(src+b),1) - 1 + relu(src+b)."""
            n = src_ap.shape[-1]
            e = pool.tile([nparts, n], BF, tag="elu_e", name="elu_e")
            r = pool.tile([nparts, n], BF, tag="elu_r", name="elu_r")
            f = pool.tile([nparts, n], BF, tag="elu_f", name="elu_f")
            nc.scalar.activation(e[:], src_ap, AF.Exp, bias=bias_tile[:],
                                 scale=1.0)
            if ASSIGN[f"r_{site}"] == "act":
                nc.scalar.activation(r[:], src_ap, AF.Relu, bias=bias_tile[:],
                                     scale=1.0)
            else:
                nc.vector.tensor_scalar(r[:], src_ap, bias_tile[:], 0.0,
                                        op0=AX.add, op1=AX.max)
            nc.vector.tensor_scalar(f[:], e[:], 1.0, -1.0,
                                    op0=AX.min, op1=AX.add)
            nc.vector.tensor_tensor(dst_ap, f[:], r[:], op=AX.add)

        # ===== PHASES A/B/C in 4 pipelined chunks (64 samples each) ==========
        # wide tiles hold two 504-token halves at bank-aligned offsets 0, LP
        H0 = slice(0, L)
        H1 = slice(LP, LP + L)
        HS = (H0, H1)
        NCHUNK = 1
        QPC = NQUAD // NCHUNK   # quads per chunk
        for ck in range(NCHUNK):
          # ---- phase A: conv/fuse/global ----
          for q in range(QPC * ck, QPC * (ck + 1)):
            ict = pA_in.tile([54, LW], BF, tag="ict", name="ict")
            nc.sync.dma_start(out=ict[:, H0], in_=P["ic"][2 * q])
            nc.sync.dma_start(out=ict[:, H1], in_=P["ic"][2 * q + 1])

            c1p = psA.tile([128, LW], F32, tag="pa", name="c1p")
            for h in (0, 1):
                nc.tensor.matmul(c1p[:, HS[h]], cs["w1bd"][:], ict[:, HS[h]],
                                 start=True, stop=True)
            a1 = pA_sb.tile([128, LW], BF, tag="a1", name="a1")
            elu1(pA_sb, a1[:], c1p[:], cs["b1d"], "conv1")

            c2p = psA.tile([128, LW], F32, tag="pa", name="c2p")
            for h in (0, 1):
                nc.tensor.matmul(c2p[:, HS[h]], cs["w2bd"][:], a1[:, HS[h]],
                                 start=True, stop=True)
            a2 = pA_sb.tile([128, LW], BF, tag="a2", name="a2")
            elu1(pA_sb, a2[:], c2p[:], cs["b2d"], "conv2")

            fp_ = psA.tile([128, LW], F32, tag="pa", name="fp_")
            for h in (0, 1):
                nc.tensor.matmul(fp_[:, HS[h]], cs["wflbd"][:], a2[:, HS[h]],
                                 start=True, stop=False)
                nc.tensor.matmul(fp_[:, HS[h]], cs["wfp2"][:], cs["pec"][:],
                                 start=False, stop=True)
            pwq = pw_tiles[q]
            elu1(pA_sb, pwq[:], fp_[:], cs["bfd"], "fuse")

            for h in (0, 1):
                j = 2 * q + h
                psl = HS[h]
                g1p = psG.tile([128, LW], F32, tag="pg", name="g1p")
                nc.tensor.matmul(g1p[:, H0], cs["g1w2"][0:64, :],
                                 pwq[0:64, psl], start=True, stop=True)
                nc.tensor.matmul(g1p[:, H1], cs["g1w2"][64:128, :],
                                 pwq[64:128, psl], start=True, stop=True)
                g1a = pA_sb.tile([128, LW], BF, tag="g1a", name="g1a")
                elu1(pA_sb, g1a[:], g1p[:], cs["bg1d"], "g1")
                g2p = psG.tile([128, LW], F32, tag="pg", name="g2p")
                for s in (0, 1):
                    nc.tensor.matmul(g2p[:, HS[s]], cs["g2w"][:],
                                     g1a[:, HS[s]], start=True, stop=True)
                for s in (0, 1):
                    sidx = 2 * j + s
                    nc.vector.tensor_reduce(
                        gf_all[:, sidx:sidx + 1], g2p[:, HS[s]],
                        axis=mybir.AxisListType.X, op=AX.max)

          # ---- phase B (chunk): global bias + q/Q projections ----
          SPC = 4 * QPC      # samples per chunk (64)
          PPC = 2 * QPC      # pairs per chunk (32)
          ss = slice(SPC * ck, SPC * (ck + 1))
          nc.vector.tensor_scalar(gf_all[:, ss], gf_all[:, ss], cs["bg2"][:],
                                  None, op0=AX.add)
          nc.vector.tensor_copy(gf_bf[:, ss], gf_all[:, ss])
          qp_ = psG.tile([D_LOCAL, SPC], F32, tag="pg", name="qp_")
          nc.tensor.matmul(qp_[:], cs["qpwg"][:], gf_bf[:, ss], start=True,
                           stop=False)
          nc.tensor.matmul(qp_[:], cs["qpwp"][:], cprop[:, ss], start=False,
                           stop=True)
          qe = pC_sb.tile([D_LOCAL, SPC], BF, tag="qe", name="qe")
          qr = pC_sb.tile([D_LOCAL, SPC], BF, tag="qr", name="qr")
          qsb = pC_sb.tile([D_LOCAL, SPC], BF, tag="qsb", name="qsb")
          nc.scalar.activation(qe[:], qp_[:], AF.Exp, bias=cs["qpb"][:],
                               scale=1.0)
          nc.scalar.activation(qr[:], qp_[:], AF.Relu, bias=cs["qpb"][:],
                               scale=1.0)
          nc.vector.tensor_scalar(qe[:], qe[:], 1.0, -1.0, op0=AX.min,
                                  op1=AX.add)
          nc.vector.tensor_tensor(qsb[:], qe[:], qr[:], op=AX.add)
          Qp = psG.tile([128, PPC], F32, tag="pg", name="Qp")
          qs_eo = qsb[:].rearrange("p (j s) -> p s j", s=2)
          nc.tensor.matmul(Qp[0:64, :], cs["wq"][:], qs_eo[:, 0, :],
                           start=True, stop=True)
          nc.tensor.matmul(Qp[64:128, :], cs["wq"][:], qs_eo[:, 1, :],
                           start=True, stop=True, tile_position=(0, 64))
          js = slice(PPC * ck, PPC * (ck + 1))
          nc.vector.tensor_scalar(Q2[:, js], Qp[:], cs["bq2"][:], None,
                                  op0=AX.add)

          # ---- phase C: attention, software-pipelined 2 pairs at a time ----
          for jj in range(PPC * ck, PPC * (ck + 1), 2):
            grp = (jj, jj + 1)
            vks_t = {}
            for j in grp:
                pwj = pw_tiles[j // 2][:, (j % 2) * LP:(j % 2) * LP + L]
                vkp = psA.tile([128, LW], F32, tag="pa", name="vkp")
                nc.tensor.matmul(vkp[:, 0:L], cs["wvbd"][:], pwj, start=True,
                                 stop=True)
                nc.tensor.matmul(vkp[:, LP:LP + L], cs["wkbd"][:], pwj,
                                 start=True, stop=True)
                vks = pC_sb.tile([128, LW], BF, tag="vks", name="vks")
                # split the evacuation across both engines
                nc.vector.tensor_copy(vks[:, 0:L], vkp[:, 0:L])
                nc.scalar.activation(vks[:, LP:LP + L], vkp[:, LP:LP + L],
                                     AF.Copy)
                vks_t[j] = vks
            for j in grp:
                vks = vks_t[j]
                sqbd = pC_sb.tile([128, 128], BF, tag="sqbd", name="sqbd")
                nc.vector.tensor_scalar(sqbd[:], cs["csmbd"][:],
                                        Q2[:, j:j + 1], None, op0=AX.mult)
                scp = psG.tile([128, LW], F32, tag="pg", name="scp")
                nc.tensor.matmul(scp[:, 0:L], sqbd[:], vks[:, LP:LP + L],
                                 start=True, stop=True)
                esb = pC_sb.tile([128, L], BF, tag="esb", name="esb")
                sume = pC_sb.tile([128, 1], F32, tag="sume", name="sume")
                nc.scalar.activation(esb[:], scp[:, 0:L], AF.Exp,
                                     accum_out=sume[:])
                rec = pC_sb.tile([128, 1], F32, tag="rec", name="rec")
                nc.vector.reciprocal(rec[:], sume[:])
                wvt = pC_sb.tile([128, L], BF, tag="wvt", name="wvt")
                ctxu = pC_sb.tile([128, 1], F32, tag="ctxu", name="ctxu")
                nc.vector.scalar_tensor_tensor(wvt[:], esb[:], 1.0,
                                               vks[:, 0:L], op0=AX.mult,
                                               op1=AX.mult, accum_out=ctxu[:])
                nc.vector.tensor_scalar(ctx_all[:, j:j + 1], ctxu[:], rec[:],
                                        cs["cvd"][:], op0=AX.mult, op1=AX.add)

        # ================= PHASE D: output projection + stores ================
        wlp = psG.tile([128, NPAIR], F32, tag="pg", name="wlp")
        nc.tensor.matmul(wlp[:], cs["wobd"][:], ctx_all[:], start=True,
                         stop=True)
        wl = singles.tile([128, NPAIR], F32, tag="wl", name="wl")
        nc.vector.tensor_scalar(wl[:], wlp[:], cs["bod"][:], None, op0=AX.add)

        OD = D_LOCAL + D_GLOBAL  # 192
        nc.sync.dma_start(
            out=dram_ap(out_h, 0, [[1, 64], [2 * OD, NPAIR]]), in_=wl[0:64, :])
        nc.sync.dma_start(
            out=dram_ap(out_h, OD, [[1, 64], [2 * OD, NPAIR]]),
            in_=wl[64:128, :])
        nc.sync.dma_start(
            out=dram_ap(out_h, 64, [[1, D_GLOBAL], [OD, B_LOC]]), in_=gf_all[:])

    _split_multiwait(nc)
    return nc


# ----------------------------------------------------------------------------
# entry point
# ----------------------------------------------------------------------------
_CACHE = {}


def kernel(**inputs):
    shared = host_prep_shared(inputs)
    cores = host_prep_percore(inputs)

    if "nc" not in _CACHE:
        _CACHE["nc"] = build_nc(shared)
    nc = _CACHE["nc"]

    in_maps = []
    for ci in range(N_CORES):
        m = dict(cores[ci])
        for name, arr in shared.items():
            m[name] = arr
        in_maps.append(m)

    trace = bool(int(os.environ.get("AME2_TRACE", "0")))
    res = run_bass_kernel_spmd(nc, in_maps, core_ids=list(range(N_CORES)),
                               trace=trace)
    if trace and res.exec_time_ns is not None:
        _CACHE["exec_time_ns"] = res.exec_time_ns
    outs = [res.results[ci]["out"] for ci in range(N_CORES)]
    return np.concatenate(outs, axis=0).astype(np.float32)


# revision 18
# speedup vs baseline: 1.1868x; 1.1868x over previous
"""Trainium2 Bass kernel for nn_AME2Encoder (dense_mlp, 8-core data parallel).

Strategy:
  - Pure data parallel: B=2048 sharded 256/core; each core processes its
    samples as 128 "pairs" (2 samples packed on the 128 SBUF partitions),
    with most elementwise work done on 2-pair-wide [128, 1008] tiles to
    amortize per-instruction overheads.
  - Feature-major bf16 activations ([feat, token] tiles, token tile = 504
    = one sample's full 14x36 grid). No transposes needed anywhere.
  - 64-feature layers are packed 2-samples-per-matmul with block-diagonal
    weights (M=128).
  - conv1 (3x3) is a single K=54 matmul per pair over a host-prepared
    im2col layout (input layout prep; all FLOPs stay on device).
  - ELU in 3 passes via the "+1 fold": every ELU site computes
    elu(x)+1 = min(exp(x+b),1) + relu(x+b); the -1 is folded into the next
    layer's bias on the host (scores are softmax-shift-invariant for K,
    and V/global-max shifts fold into constants).
      pass1: ACT Exp(psum + bias) -> e (bf16)
      pass2: ACT Relu(psum + bias) or DVE tensor_scalar -> r (bf16)
      pass3: scalar_tensor_tensor (e min 1) add r -> out (one fused pass)
  - Attention: block-diag 0.5*Q masks -> one scores matmul per pair; ACT
    exp with free sum accumulation; fused V-weighting + context reduction
    in one scalar_tensor_tensor with accum_out.
  - K/V biases: bk drops out of softmax; bv folded into the context
    normalization. So K|V evacuation is a single wide pure-copy pass.
"""

import os
from contextlib import ExitStack

import numpy as np
import ml_dtypes

import concourse.bass as bass
import concourse.mybir as mybir
import concourse.tile as tile
from concourse.bass_utils import run_bass_kernel_spmd
from concourse.vector_clock import ScopedClock


# --- workaround: this walrus rejects the tail Drain carrying >1 sem waits ---
def _patched_dab(self, tick_clock, wait_clock):
    nc = self.nc
    probe = nc.sync.drain()
    wait_clock.add_sem_waits(probe.ins, ScopedClock({None: tick_clock.global_clock}))
    si = probe.ins.sync_info
    waits = list(si.on_wait) if si is not None else []
    if si is not None and len(waits) > 1:
        si.on_wait = waits[:1]
        for w in waits[1:]:
            n2 = nc.sync.drain()
            n2.ins.sync_info = mybir.SyncInfo(on_wait=[w], on_update=[])
    nc.all_engine_barrier()
    assert self.sems is not None
    popped = nc._tile_sem_poison_stack.pop()
    assert popped is self._sem_poison
    nc.clear_and_free_semaphores(list(self.sems.allocated().values()))
    nc.all_engine_barrier()


tile.TileContext._drain_and_barrier = _patched_dab


def _split_multiwait(nc, max_waits=1):
    """This walrus build cannot encode >1 sem-wait on one instruction for some
    structs; hoist excess waits onto EventSemaphore carriers inserted before."""
    ctr = [0]
    for fn in nc.m.functions:
        for blk in fn.blocks:
            insts = list(blk.instructions)
            new = []
            changed = False
            for inst in insts:
                si = inst.sync_info
                waits = list(si.on_wait) if si is not None and si.on_wait else []
                if len(waits) > max_waits:
                    changed = True
                    for w in waits[max_waits:]:
                        ctr[0] += 1
                        new.append(mybir.InstEventSemaphore(
                            name=f"zz_mw_{ctr[0]}", engine=inst.engine,
                            ins=[], outs=[],
                            sync_info=mybir.SyncInfo(on_wait=[w], on_update=[]),
                        ))
                    inst.sync_info = mybir.SyncInfo(
                        on_wait=waits[:max_waits],
                        on_update=list(si.on_update) if si.on_update else [],
                    )
                new.append(inst)
            if changed:
                blk.instructions = new


# ----- problem constants (hardcoded per spec) -----
B, C_IN, H, W = 2048, 3, 14, 36
D_LOCAL, D_POS, D_GLOBAL, D_PROP, NH = 64, 64, 128, 128, 16
HD = D_LOCAL // NH
N_CORES = 8
B_LOC = B // N_CORES      # 256
NPAIR = B_LOC // 2        # 128
NQUAD = NPAIR // 2        # 64 wide iterations (2 pairs each)
L = H * W                 # 504
LP = 512                  # PSUM-bank-aligned half stride
LW = LP + L               # 1016: wide tile width (second half at [LP, LP+L))

BF = mybir.dt.float16
F32 = mybir.dt.float32
bf16 = np.float16
AX = mybir.AluOpType
AF = mybir.ActivationFunctionType

# engine assignment for the flexible passes ("act" or "dve"); stt passes may
# also go to "gp" (gpsimd).
ASSIGN = {
    "r_conv1": "act", "r_conv2": "act", "r_fuse": "act", "r_g1": "dve",
    "kvevac": "act",
    "stt_conv1": "dve", "stt_conv2": "dve", "stt_fuse": "dve", "stt_g1": "dve",
}


def _np_elu(x):
    return np.where(x > 0, x, np.expm1(np.minimum(x, 0.0)))


# ----------------------------------------------------------------------------
# Host-side constant packing (weight folding / layout prep)
# ----------------------------------------------------------------------------

def _block_diag2(w):
    k, m = w.shape
    out = np.zeros((2 * k, 2 * m), np.float32)
    out[:k, :m] = w
    out[k:, m:] = w
    return out


def _dup_col(b):
    return np.concatenate([b, b]).astype(np.float32)[:, None]


def host_prep_shared(inp):
    """Weight-derived dram parameters. All ELU sites produce elu(x)+1; the -1
    is folded into each consumer's bias here (colsum of the consumer weight)."""
    c = {}
    w1p = inp["conv1_w"].transpose(2, 3, 1, 0).reshape(27, 64)  # k=(3dy+dx)*3+c
    c["w1bd"] = _block_diag2(w1p).astype(bf16)            # [54,128]
    c["b1d"] = _dup_col(inp["conv1_b"])                   # [128,1] f32

    w2 = inp["conv2_w"][:, :, 0, 0].T                     # [in,out]
    c["w2bd"] = _block_diag2(w2).astype(bf16)             # [128,128]
    c["b2d"] = _dup_col(inp["conv2_b"])

    fl = inp["fuse_w"][:D_LOCAL]                          # [64,64]
    fp = inp["fuse_w"][D_LOCAL:]                          # [64,64]
    c["wflbd"] = _block_diag2(fl).astype(bf16)            # [128,128]
    c["wfp2"] = np.concatenate([fp, fp], axis=1).astype(bf16)  # [64,128]
    c["bfd"] = _dup_col(inp["fuse_b"])

    ys = np.linspace(-1.0, 1.0, H, dtype=np.float32)
    xs = np.linspace(-1.0, 1.0, W, dtype=np.float32)
    gy, gx = np.meshgrid(ys, xs, indexing="ij")
    coords = np.stack([gx, gy], axis=-1).reshape(L, 2)
    pe = _np_elu(coords @ inp["pe_w1"] + inp["pe_b1"]) @ inp["pe_w2"] + inp["pe_b2"]
    c["pec"] = np.ascontiguousarray(pe.T).astype(bf16)    # [64,504] exact

    g1 = inp["g_w1"]                                      # [64,128]
    c["g1w2"] = np.vstack([g1, g1]).astype(bf16)          # [128,128] dup rows
    c["bg1d"] = inp["g_b1"].astype(np.float32)[:, None]

    g2 = inp["g_w2"]
    c["g2w"] = g2.astype(bf16)                            # [128,128]
    c["bg2"] = inp["g_b2"].astype(np.float32)[:, None]    # applied post-gmax

    c["wvbd"] = _block_diag2(inp["wv"]).astype(bf16)      # [128,128]
    c["wkbd"] = _block_diag2(inp["wk"]).astype(bf16)      # bk drops in softmax
    c["cvd"] = _dup_col(inp["bv"])  # folded into ctx normalization

    sm = np.zeros((64, 64), np.float32)
    for k in range(64):
        sm[k, (k // HD) * HD:(k // HD + 1) * HD] = 1.0 / np.sqrt(HD)
    c["csmbd"] = _block_diag2(sm).astype(bf16)            # [128,128]

    c["qpwg"] = inp["qp_w"][:D_GLOBAL].astype(bf16)       # [128,64]
    c["qpwp"] = inp["qp_w"][D_GLOBAL:].astype(bf16)       # [128,64]
    c["qpb"] = inp["qp_b"].astype(np.float32)[:, None]    # [64,1]

    c["wq"] = inp["wq"].astype(bf16)
    c["bq2"] = _dup_col(inp["bq"])                        # [128,1]

    c["wobd"] = _block_diag2(inp["wo"]).astype(bf16)      # [128,128]
    c["bod"] = _dup_col(inp["bo"])
    return c


def host_prep_percore(inp):
    """Per-core data params: im2col input layout + transposed prop_emb."""
    mf = inp["map_feat"].astype(np.float32)
    mp = np.zeros((B, 3, H + 2, W + 2), np.float32)
    mp[:, :, 1:H + 1, 1:W + 1] = mf
    from numpy.lib.stride_tricks import sliding_window_view
    sw = sliding_window_view(mp, (3, 3), axis=(2, 3))     # [B,3,14,36,3,3]
    ic = sw.transpose(0, 4, 5, 1, 2, 3).reshape(B, 27, L)  # k=(3dy+dx)*3+c
    ic = np.ascontiguousarray(ic).reshape(B // 2, 54, L).astype(bf16)
    prop = inp["prop_emb"].astype(np.float32)
    cores = []
    for ci in range(N_CORES):
        sl = slice(ci * B_LOC, (ci + 1) * B_LOC)
        cores.append({
            "ic": np.ascontiguousarray(ic[ci * NPAIR:(ci + 1) * NPAIR]),
            "propT": np.ascontiguousarray(prop[sl].T).astype(bf16),  # [128,256]
        })
    return cores


# ----------------------------------------------------------------------------
# Bass graph
# ----------------------------------------------------------------------------

def build_nc(shared):
    nc = bass.Bass()

    P = {}
    P["ic"] = nc.declare_dram_parameter("ic", [NPAIR, 54, L], BF, isOutput=False)
    P["propT"] = nc.declare_dram_parameter("propT", [D_PROP, B_LOC], BF,
                                           isOutput=False)
    for name, arr in shared.items():
        dt = BF if arr.dtype == bf16 else F32
        P[name] = nc.declare_dram_parameter(name, list(arr.shape), dt,
                                            isOutput=False)
    out_h = nc.declare_dram_parameter("out", [B_LOC, D_LOCAL + D_GLOBAL], F32,
                                      isOutput=True)

    def dram_ap(h, offset, dims):
        base = h[:]
        return bass.AP(tensor=base.tensor, offset=offset,
                       ap=[list(d) for d in dims])

    with tile.TileContext(nc) as tc, ExitStack() as ctx:
        singles = ctx.enter_context(tc.tile_pool(name="singles", bufs=1))
        pA_in = ctx.enter_context(tc.tile_pool(name="pA_in", bufs=4))
        pA_sb = ctx.enter_context(tc.tile_pool(name="pA_sb", bufs=2))
        pC_sb = ctx.enter_context(tc.tile_pool(name="pC_sb", bufs=2))
        psA = ctx.enter_context(tc.tile_pool(name="psA", bufs=2, space="PSUM"))
        psG = ctx.enter_context(tc.tile_pool(name="psG", bufs=2, space="PSUM"))

        # ---- constants ----
        cs = {}
        for name, arr in shared.items():
            dt = BF if arr.dtype == bf16 else F32
            t = singles.tile(list(arr.shape), dt, tag=f"c_{name}",
                             name=f"c_{name}")
            nc.sync.dma_start(out=t[:], in_=P[name][:])
            cs[name] = t
        cprop = singles.tile([D_PROP, B_LOC], BF, tag="c_prop", name="c_prop")
        nc.sync.dma_start(out=cprop[:], in_=P["propT"][:])

        # persistent state
        gf_all = singles.tile([D_GLOBAL, B_LOC], F32, tag="gf_all", name="gf_all")
        gf_bf = singles.tile([D_GLOBAL, B_LOC], BF, tag="gf_bf", name="gf_bf")
        ctx_all = singles.tile([128, NPAIR], BF, tag="ctx_all", name="ctx_all")
        Q2 = singles.tile([128, NPAIR], F32, tag="Q2", name="Q2")
        pw_tiles = [singles.tile([128, LW], BF, tag=f"pw{q}", name=f"pw{q}")
                    for q in range(NQUAD)]

        ENG = {"act": nc.scalar, "dve": nc.vector, "gp": nc.gpsimd}

        def elu1(pool, dst_ap, src_ap, bias_tile, site, nparts=128):
            """dst = elu(src + b) = min(exp(src+b),1) - 1 + relu(src+b)."""
            n = src_ap.shape[-1]
            e = pool.tile([nparts, n], BF, tag="elu_e", name="elu_e")
            r = pool.tile([nparts, n], BF, tag="elu_r", name="elu_r")
            f = pool.tile([nparts, n], BF, tag="elu_f", name="elu_f")
            nc.scalar.activation(e[:], src_ap, AF.Exp, bias=bias_tile[:],
                                 scale=1.0)
            if ASSIGN[f"r_{site}"] == "act":
                nc.scalar.activation(r[:], src_ap, AF.Relu, bias=bias_tile[:],
                                     scale=1.0)
            else:
                nc.vector.tensor_scalar(r[:], src_ap, bias_tile[:], 0.0,
                                        op0=AX.add, op1=AX.max)
            nc.vector.tensor_scalar(f[:], e[:], 1.0, -1.0,
                                    op0=AX.min, op1=AX.add)
            nc.vector.tensor_tensor(dst_ap, f[:], r[:], op=AX.add)

        # ===== PHASES A/B/C in 4 pipelined chunks (64 samples each) ==========
        # wide tiles hold two 504-token halves at bank-aligned offsets 0, LP
        H0 = slice(0, L)
        H1 = slice(LP, LP + L)
        HS = (H0, H1)
        NCHUNK = 4
        QPC = NQUAD // NCHUNK   # quads per chunk
        for ck in range(NCHUNK):
          # ---- phase A: conv/fuse/global ----
          for q in range(QPC * ck, QPC * (ck + 1)):
            ict = pA_in.tile([54, LW], BF, tag="ict", name="ict")
            nc.sync.dma_start(out=ict[:, H0], in_=P["ic"][2 * q])
            nc.sync.dma_start(out=ict[:, H1], in_=P["ic"][2 * q + 1])

            c1p = psA.tile([128, LW], F32, tag="pa", name="c1p")
            for h in (0, 1):
                nc.tensor.matmul(c1p[:, HS[h]], cs["w1bd"][:], ict[:, HS[h]],
                                 start=True, stop=True)
            a1 = pA_sb.tile([128, LW], BF, tag="a1", name="a1")
            elu1(pA_sb, a1[:], c1p[:], cs["b1d"], "conv1")

            c2p = psA.tile([128, LW], F32, tag="pa", name="c2p")
            for h in (0, 1):
                nc.tensor.matmul(c2p[:, HS[h]], cs["w2bd"][:], a1[:, HS[h]],
                                 start=True, stop=True)
            a2 = pA_sb.tile([128, LW], BF, tag="a2", name="a2")
            elu1(pA_sb, a2[:], c2p[:], cs["b2d"], "conv2")

            fp_ = psA.tile([128, LW], F32, tag="pa", name="fp_")
            for h in (0, 1):
                nc.tensor.matmul(fp_[:, HS[h]], cs["wflbd"][:], a2[:, HS[h]],
                                 start=True, stop=False)
                nc.tensor.matmul(fp_[:, HS[h]], cs["wfp2"][:], cs["pec"][:],
                                 start=False, stop=True)
            pwq = pw_tiles[q]
            elu1(pA_sb, pwq[:], fp_[:], cs["bfd"], "fuse")

            for h in (0, 1):
                j = 2 * q + h
                psl = HS[h]
                g1p = psG.tile([128, LW], F32, tag="pg", name="g1p")
                nc.tensor.matmul(g1p[:, H0], cs["g1w2"][0:64, :],
                                 pwq[0:64, psl], start=True, stop=True)
                nc.tensor.matmul(g1p[:, H1], cs["g1w2"][64:128, :],
                                 pwq[64:128, psl], start=True, stop=True)
                g1a = pA_sb.tile([128, LW], BF, tag="g1a", name="g1a")
                elu1(pA_sb, g1a[:], g1p[:], cs["bg1d"], "g1")
                g2p = psG.tile([128, LW], F32, tag="pg", name="g2p")
                for s in (0, 1):
                    nc.tensor.matmul(g2p[:, HS[s]], cs["g2w"][:],
                                     g1a[:, HS[s]], start=True, stop=True)
                for s in (0, 1):
                    sidx = 2 * j + s
                    nc.vector.tensor_reduce(
                        gf_all[:, sidx:sidx + 1], g2p[:, HS[s]],
                        axis=mybir.AxisListType.X, op=AX.max)

          # ---- phase B (chunk): global bias + q/Q projections ----
          SPC = 4 * QPC      # samples per chunk (64)
          PPC = 2 * QPC      # pairs per chunk (32)
          ss = slice(SPC * ck, SPC * (ck + 1))
          nc.vector.tensor_scalar(gf_all[:, ss], gf_all[:, ss], cs["bg2"][:],
                                  None, op0=AX.add)
          nc.vector.tensor_copy(gf_bf[:, ss], gf_all[:, ss])
          qp_ = psG.tile([D_LOCAL, SPC], F32, tag="pg", name="qp_")
          nc.tensor.matmul(qp_[:], cs["qpwg"][:], gf_bf[:, ss], start=True,
                           stop=False)
          nc.tensor.matmul(qp_[:], cs["qpwp"][:], cprop[:, ss], start=False,
                           stop=True)
          qe = pC_sb.tile([D_LOCAL, SPC], BF, tag="qe", name="qe")
          qr = pC_sb.tile([D_LOCAL, SPC], BF, tag="qr", name="qr")
          qsb = pC_sb.tile([D_LOCAL, SPC], BF, tag="qsb", name="qsb")
          nc.scalar.activation(qe[:], qp_[:], AF.Exp, bias=cs["qpb"][:],
                               scale=1.0)
          nc.scalar.activation(qr[:], qp_[:], AF.Relu, bias=cs["qpb"][:],
                               scale=1.0)
          nc.vector.tensor_scalar(qe[:], qe[:], 1.0, -1.0, op0=AX.min,
                                  op1=AX.add)
          nc.vector.tensor_tensor(qsb[:], qe[:], qr[:], op=AX.add)
          Qp = psG.tile([128, PPC], F32, tag="pg", name="Qp")
          qs_eo = qsb[:].rearrange("p (j s) -> p s j", s=2)
          nc.tensor.matmul(Qp[0:64, :], cs["wq"][:], qs_eo[:, 0, :],
                           start=True, stop=True)
          nc.tensor.matmul(Qp[64:128, :], cs["wq"][:], qs_eo[:, 1, :],
                           start=True, stop=True, tile_position=(0, 64))
          js = slice(PPC * ck, PPC * (ck + 1))
          nc.vector.tensor_scalar(Q2[:, js], Qp[:], cs["bq2"][:], None,
                                  op0=AX.add)

          # ---- phase C: attention per pair ----
          for j in range(PPC * ck, PPC * (ck + 1)):
            pwj = pw_tiles[j // 2][:, (j % 2) * LP:(j % 2) * LP + L]
            vkp = psA.tile([128, LW], F32, tag="pa", name="vkp")
            nc.tensor.matmul(vkp[:, 0:L], cs["wvbd"][:], pwj, start=True,
                             stop=True)
            nc.tensor.matmul(vkp[:, LP:LP + L], cs["wkbd"][:], pwj, start=True,
                             stop=True)
            vks = pC_sb.tile([128, LW], BF, tag="vks", name="vks")
            nc.scalar.activation(vks[:], vkp[:], AF.Copy)
            sqbd = pC_sb.tile([128, 128], BF, tag="sqbd", name="sqbd")
            nc.vector.tensor_scalar(sqbd[:], cs["csmbd"][:], Q2[:, j:j + 1],
                                    None, op0=AX.mult)
            scp = psG.tile([128, LW], F32, tag="pg", name="scp")
            nc.tensor.matmul(scp[:, 0:L], sqbd[:], vks[:, LP:LP + L],
                             start=True, stop=True)
            esb = pC_sb.tile([128, L], BF, tag="esb", name="esb")
            sume = pC_sb.tile([128, 1], F32, tag="sume", name="sume")
            nc.scalar.activation(esb[:], scp[:, 0:L], AF.Exp,
                                 accum_out=sume[:])
            rec = pC_sb.tile([128, 1], F32, tag="rec", name="rec")
            nc.vector.reciprocal(rec[:], sume[:])
            wvt = pC_sb.tile([128, L], BF, tag="wvt", name="wvt")
            ctxu = pC_sb.tile([128, 1], F32, tag="ctxu", name="ctxu")
            nc.vector.scalar_tensor_tensor(wvt[:], esb[:], 1.0, vks[:, 0:L],
                                           op0=AX.mult, op1=AX.mult,
                                           accum_out=ctxu[:])
            nc.vector.tensor_scalar(ctx_all[:, j:j + 1], ctxu[:], rec[:],
                                    cs["cvd"][:], op0=AX.mult, op1=AX.add)

        # ================= PHASE D: output projection + stores ================
        wlp = psG.tile([128, NPAIR], F32, tag="pg", name="wlp")
        nc.tensor.matmul(wlp[:], cs["wobd"][:], ctx_all[:], start=True,
                         stop=True)
        wl = singles.tile([128, NPAIR], F32, tag="wl", name="wl")
        nc.vector.tensor_scalar(wl[:], wlp[:], cs["bod"][:], None, op0=AX.add)

        OD = D_LOCAL + D_GLOBAL  # 192
        nc.sync.dma_start(
            out=dram_ap(out_h, 0, [[1, 64], [2 * OD, NPAIR]]), in_=wl[0:64, :])
        nc.sync.dma_start(
            out=dram_ap(out_h, OD, [[1, 64], [2 * OD, NPAIR]]),
            in_=wl[64:128, :])
        nc.sync.dma_start(
            out=dram_ap(out_h, 64, [[1, D_GLOBAL], [OD, B_LOC]]), in_=gf_all[:])

    _split_multiwait(nc)
    return nc


# ----------------------------------------------------------------------------
# entry point
# ----------------------------------------------------------------------------
_CACHE = {}


def kernel(**inputs):
    shared = host_prep_shared(inputs)
    cores = host_prep_percore(inputs)

    if "nc" not in _CACHE:
        _CACHE["nc"] = build_nc(shared)
    nc = _CACHE["nc"]

    in_maps = []
    for ci in range(N_CORES):
        m = dict(cores[ci])
        for name, arr in shared.items():
            m[name] = arr
        in_maps.append(m)

    trace = bool(int(os.environ.get("AME2_TRACE", "0")))
    res = run_bass_kernel_spmd(nc, in_maps, core_ids=list(range(N_CORES)),
                               trace=trace)
    if trace and res.exec_time_ns is not None:
        _CACHE["exec_time_ns"] = res.exec_time_ns
    outs = [res.results[ci]["out"] for ci in range(N_CORES)]
    return np.concatenate(outs, axis=0).astype(np.float32)


# revision 19
# speedup vs baseline: 1.4703x; 1.2388x over previous
"""Trainium2 Bass kernel for nn_AME2Encoder (dense_mlp, 8-core data parallel).

Strategy:
  - Pure data parallel: B=2048 sharded 256/core; each core processes its
    samples as 128 "pairs" (2 samples packed on the 128 SBUF partitions),
    with most elementwise work done on 2-pair-wide [128, 1008] tiles to
    amortize per-instruction overheads.
  - Feature-major bf16 activations ([feat, token] tiles, token tile = 504
    = one sample's full 14x36 grid). No transposes needed anywhere.
  - 64-feature layers are packed 2-samples-per-matmul with block-diagonal
    weights (M=128).
  - conv1 (3x3) is a single K=54 matmul per pair over a host-prepared
    im2col layout (input layout prep; all FLOPs stay on device).
  - ELU in 3 passes via the "+1 fold": every ELU site computes
    elu(x)+1 = min(exp(x+b),1) + relu(x+b); the -1 is folded into the next
    layer's bias on the host (scores are softmax-shift-invariant for K,
    and V/global-max shifts fold into constants).
      pass1: ACT Exp(psum + bias) -> e (bf16)
      pass2: ACT Relu(psum + bias) or DVE tensor_scalar -> r (bf16)
      pass3: scalar_tensor_tensor (e min 1) add r -> out (one fused pass)
  - Attention: block-diag 0.5*Q masks -> one scores matmul per pair; ACT
    exp with free sum accumulation; fused V-weighting + context reduction
    in one scalar_tensor_tensor with accum_out.
  - K/V biases: bk drops out of softmax; bv folded into the context
    normalization. So K|V evacuation is a single wide pure-copy pass.
"""

import os
from contextlib import ExitStack

import numpy as np
import ml_dtypes

import concourse.bass as bass
import concourse.mybir as mybir
import concourse.tile as tile
from concourse.bass_utils import run_bass_kernel_spmd
from concourse.vector_clock import ScopedClock


# --- workaround: this walrus rejects the tail Drain carrying >1 sem waits ---
def _patched_dab(self, tick_clock, wait_clock):
    nc = self.nc
    probe = nc.sync.drain()
    wait_clock.add_sem_waits(probe.ins, ScopedClock({None: tick_clock.global_clock}))
    si = probe.ins.sync_info
    waits = list(si.on_wait) if si is not None else []
    if si is not None and len(waits) > 1:
        si.on_wait = waits[:1]
        for w in waits[1:]:
            n2 = nc.sync.drain()
            n2.ins.sync_info = mybir.SyncInfo(on_wait=[w], on_update=[])
    nc.all_engine_barrier()
    assert self.sems is not None
    popped = nc._tile_sem_poison_stack.pop()
    assert popped is self._sem_poison
    nc.clear_and_free_semaphores(list(self.sems.allocated().values()))
    nc.all_engine_barrier()


tile.TileContext._drain_and_barrier = _patched_dab


def _split_multiwait(nc, max_waits=1):
    """This walrus build cannot encode >1 sem-wait on one instruction for some
    structs; hoist excess waits onto EventSemaphore carriers inserted before."""
    ctr = [0]
    for fn in nc.m.functions:
        for blk in fn.blocks:
            insts = list(blk.instructions)
            new = []
            changed = False
            for inst in insts:
                si = inst.sync_info
                waits = list(si.on_wait) if si is not None and si.on_wait else []
                if len(waits) > max_waits:
                    changed = True
                    for w in waits[max_waits:]:
                        ctr[0] += 1
                        new.append(mybir.InstEventSemaphore(
                            name=f"zz_mw_{ctr[0]}", engine=inst.engine,
                            ins=[], outs=[],
                            sync_info=mybir.SyncInfo(on_wait=[w], on_update=[]),
                        ))
                    inst.sync_info = mybir.SyncInfo(
                        on_wait=waits[:max_waits],
                        on_update=list(si.on_update) if si.on_update else [],
                    )
                new.append(inst)
            if changed:
                blk.instructions = new


# ----- problem constants (hardcoded per spec) -----
B, C_IN, H, W = 2048, 3, 14, 36
D_LOCAL, D_POS, D_GLOBAL, D_PROP, NH = 64, 64, 128, 128, 16
HD = D_LOCAL // NH
N_CORES = 8
B_LOC = B // N_CORES      # 256
NPAIR = B_LOC // 2        # 128
NQUAD = NPAIR // 2        # 64 wide iterations (2 pairs each)
L = H * W                 # 504
LP = 512                  # PSUM-bank-aligned half stride
LW = LP + L               # 1016: wide tile width (second half at [LP, LP+L))

BF = mybir.dt.float16
F32 = mybir.dt.float32
bf16 = np.float16
AX = mybir.AluOpType
AF = mybir.ActivationFunctionType

# engine assignment for the flexible passes ("act" or "dve"); stt passes may
# also go to "gp" (gpsimd).
ASSIGN = {
    "r_conv1": "act", "r_conv2": "act", "r_fuse": "act", "r_g1": "dve",
    "kvevac": "act",
    "stt_conv1": "dve", "stt_conv2": "dve", "stt_fuse": "dve", "stt_g1": "dve",
}


def _np_elu(x):
    return np.where(x > 0, x, np.expm1(np.minimum(x, 0.0)))


# ----------------------------------------------------------------------------
# Host-side constant packing (weight folding / layout prep)
# ----------------------------------------------------------------------------

def _block_diag2(w):
    k, m = w.shape
    out = np.zeros((2 * k, 2 * m), np.float32)
    out[:k, :m] = w
    out[k:, m:] = w
    return out


def _dup_col(b):
    return np.concatenate([b, b]).astype(np.float32)[:, None]


def host_prep_shared(inp):
    """Weight-derived dram parameters. All ELU sites produce elu(x)+1; the -1
    is folded into each consumer's bias here (colsum of the consumer weight)."""
    c = {}
    w1p = inp["conv1_w"].transpose(2, 3, 1, 0).reshape(27, 64)  # k=(3dy+dx)*3+c
    c["w1bd"] = _block_diag2(w1p).astype(bf16)            # [54,128]
    c["b1d"] = _dup_col(inp["conv1_b"])                   # [128,1] f32

    w2 = inp["conv2_w"][:, :, 0, 0].T                     # [in,out]
    c["w2bd"] = _block_diag2(w2).astype(bf16)             # [128,128]
    c["b2d"] = _dup_col(inp["conv2_b"])

    fl = inp["fuse_w"][:D_LOCAL]                          # [64,64]
    fp = inp["fuse_w"][D_LOCAL:]                          # [64,64]
    c["wflbd"] = _block_diag2(fl).astype(bf16)            # [128,128]
    c["wfp2"] = np.concatenate([fp, fp], axis=1).astype(bf16)  # [64,128]
    c["bfd"] = _dup_col(inp["fuse_b"])

    ys = np.linspace(-1.0, 1.0, H, dtype=np.float32)
    xs = np.linspace(-1.0, 1.0, W, dtype=np.float32)
    gy, gx = np.meshgrid(ys, xs, indexing="ij")
    coords = np.stack([gx, gy], axis=-1).reshape(L, 2)
    pe = _np_elu(coords @ inp["pe_w1"] + inp["pe_b1"]) @ inp["pe_w2"] + inp["pe_b2"]
    c["pec"] = np.ascontiguousarray(pe.T).astype(bf16)    # [64,504] exact

    g1 = inp["g_w1"]                                      # [64,128]
    c["g1w2"] = np.vstack([g1, g1]).astype(bf16)          # [128,128] dup rows
    c["bg1d"] = inp["g_b1"].astype(np.float32)[:, None]

    g2 = inp["g_w2"]
    c["g2w"] = g2.astype(bf16)                            # [128,128]
    c["bg2"] = inp["g_b2"].astype(np.float32)[:, None]    # applied post-gmax

    c["wvbd"] = _block_diag2(inp["wv"]).astype(bf16)      # [128,128]
    c["wkbd"] = _block_diag2(inp["wk"]).astype(bf16)      # bk drops in softmax
    c["cvd"] = _dup_col(inp["bv"])  # folded into ctx normalization

    sm = np.zeros((64, 64), np.float32)
    for k in range(64):
        sm[k, (k // HD) * HD:(k // HD + 1) * HD] = 1.0 / np.sqrt(HD)
    c["csmbd"] = _block_diag2(sm).astype(bf16)            # [128,128]

    c["qpwg"] = inp["qp_w"][:D_GLOBAL].astype(bf16)       # [128,64]
    c["qpwp"] = inp["qp_w"][D_GLOBAL:].astype(bf16)       # [128,64]
    c["qpb"] = inp["qp_b"].astype(np.float32)[:, None]    # [64,1]

    c["wq"] = inp["wq"].astype(bf16)
    c["bq2"] = _dup_col(inp["bq"])                        # [128,1]

    c["wobd"] = _block_diag2(inp["wo"]).astype(bf16)      # [128,128]
    c["bod"] = _dup_col(inp["bo"])
    return c


def host_prep_percore(inp):
    """Per-core data params: im2col input layout + transposed prop_emb."""
    mf = inp["map_feat"].astype(np.float32)
    mp = np.zeros((B, 3, H + 2, W + 2), np.float32)
    mp[:, :, 1:H + 1, 1:W + 1] = mf
    from numpy.lib.stride_tricks import sliding_window_view
    sw = sliding_window_view(mp, (3, 3), axis=(2, 3))     # [B,3,14,36,3,3]
    ic = sw.transpose(0, 4, 5, 1, 2, 3).reshape(B, 27, L)  # k=(3dy+dx)*3+c
    ic = np.ascontiguousarray(ic).reshape(B // 2, 54, L).astype(bf16)
    prop = inp["prop_emb"].astype(np.float32)
    cores = []
    for ci in range(N_CORES):
        sl = slice(ci * B_LOC, (ci + 1) * B_LOC)
        cores.append({
            "ic": np.ascontiguousarray(ic[ci * NPAIR:(ci + 1) * NPAIR]),
            "propT": np.ascontiguousarray(prop[sl].T).astype(bf16),  # [128,256]
        })
    return cores


# ----------------------------------------------------------------------------
# Bass graph
# ----------------------------------------------------------------------------

def build_nc(shared):
    nc = bass.Bass()

    P = {}
    P["ic"] = nc.declare_dram_parameter("ic", [NPAIR, 54, L], BF, isOutput=False)
    P["propT"] = nc.declare_dram_parameter("propT", [D_PROP, B_LOC], BF,
                                           isOutput=False)
    for name, arr in shared.items():
        dt = BF if arr.dtype == bf16 else F32
        P[name] = nc.declare_dram_parameter(name, list(arr.shape), dt,
                                            isOutput=False)
    out_h = nc.declare_dram_parameter("out", [B_LOC, D_LOCAL + D_GLOBAL], F32,
                                      isOutput=True)

    def dram_ap(h, offset, dims):
        base = h[:]
        return bass.AP(tensor=base.tensor, offset=offset,
                       ap=[list(d) for d in dims])

    with tile.TileContext(nc) as tc, ExitStack() as ctx:
        singles = ctx.enter_context(tc.tile_pool(name="singles", bufs=1))
        pA_in = ctx.enter_context(tc.tile_pool(name="pA_in", bufs=4))
        pA_sb = ctx.enter_context(tc.tile_pool(name="pA_sb", bufs=2))
        pC_sb = ctx.enter_context(tc.tile_pool(name="pC_sb", bufs=2))
        psA = ctx.enter_context(tc.tile_pool(name="psA", bufs=2, space="PSUM"))
        psG = ctx.enter_context(tc.tile_pool(name="psG", bufs=2, space="PSUM"))

        # ---- constants ----
        cs = {}
        for name, arr in shared.items():
            dt = BF if arr.dtype == bf16 else F32
            t = singles.tile(list(arr.shape), dt, tag=f"c_{name}",
                             name=f"c_{name}")
            nc.sync.dma_start(out=t[:], in_=P[name][:])
            cs[name] = t
        cprop = singles.tile([D_PROP, B_LOC], BF, tag="c_prop", name="c_prop")
        nc.sync.dma_start(out=cprop[:], in_=P["propT"][:])

        # persistent state
        gf_all = singles.tile([D_GLOBAL, B_LOC], F32, tag="gf_all", name="gf_all")
        gf_bf = singles.tile([D_GLOBAL, B_LOC], BF, tag="gf_bf", name="gf_bf")
        ctx_all = singles.tile([128, NPAIR], BF, tag="ctx_all", name="ctx_all")
        Q2 = singles.tile([128, NPAIR], F32, tag="Q2", name="Q2")
        pw_tiles = [singles.tile([128, LW], BF, tag=f"pw{q}", name=f"pw{q}")
                    for q in range(NQUAD)]

        ENG = {"act": nc.scalar, "dve": nc.vector, "gp": nc.gpsimd}

        def elu1(pool, dst_ap, src_ap, bias_tile, site, nparts=128):
            """dst = elu(src + b) = min(exp(src+b),1) - 1 + relu(src+b)."""
            n = src_ap.shape[-1]
            e = pool.tile([nparts, n], BF, tag="elu_e", name="elu_e")
            r = pool.tile([nparts, n], BF, tag="elu_r", name="elu_r")
            f = pool.tile([nparts, n], BF, tag="elu_f", name="elu_f")
            nc.scalar.activation(e[:], src_ap, AF.Exp, bias=bias_tile[:],
                                 scale=1.0)
            if ASSIGN[f"r_{site}"] == "act":
                nc.scalar.activation(r[:], src_ap, AF.Relu, bias=bias_tile[:],
                                     scale=1.0)
            else:
                nc.vector.tensor_scalar(r[:], src_ap, bias_tile[:], 0.0,
                                        op0=AX.add, op1=AX.max)
            nc.vector.tensor_scalar(f[:], e[:], 1.0, -1.0,
                                    op0=AX.min, op1=AX.add)
            nc.vector.tensor_tensor(dst_ap, f[:], r[:], op=AX.add)

        # ===== PHASES A/B/C in 4 pipelined chunks (64 samples each) ==========
        # wide tiles hold two 504-token halves at bank-aligned offsets 0, LP
        H0 = slice(0, L)
        H1 = slice(LP, LP + L)
        HS = (H0, H1)
        NCHUNK = 1
        QPC = NQUAD // NCHUNK   # quads per chunk
        for ck in range(NCHUNK):
          # ---- phase A: conv/fuse/global, 2 quads stage-interleaved ----
          for qq in range(QPC * ck, QPC * (ck + 1), 2):
            QS = (qq, qq + 1)
            ic_t, a_t = {}, {}
            for q in QS:
                ict = pA_in.tile([54, LW], BF, tag="ict", name="ict")
                nc.sync.dma_start(out=ict[:, H0], in_=P["ic"][2 * q])
                nc.sync.dma_start(out=ict[:, H1], in_=P["ic"][2 * q + 1])
                ic_t[q] = ict
            ps_t = {}
            for q in QS:
                c1p = psA.tile([128, LW], F32, tag="pa", name="c1p")
                for h in (0, 1):
                    nc.tensor.matmul(c1p[:, HS[h]], cs["w1bd"][:],
                                     ic_t[q][:, HS[h]], start=True, stop=True)
                ps_t[q] = c1p
            for q in QS:
                a1 = pA_sb.tile([128, LW], BF, tag="a1", name="a1")
                elu1(pA_sb, a1[:], ps_t[q][:], cs["b1d"], "conv1")
                a_t[q] = a1
            for q in QS:
                c2p = psA.tile([128, LW], F32, tag="pa", name="c2p")
                for h in (0, 1):
                    nc.tensor.matmul(c2p[:, HS[h]], cs["w2bd"][:],
                                     a_t[q][:, HS[h]], start=True, stop=True)
                ps_t[q] = c2p
            for q in QS:
                a2 = pA_sb.tile([128, LW], BF, tag="a2", name="a2")
                elu1(pA_sb, a2[:], ps_t[q][:], cs["b2d"], "conv2")
                a_t[q] = a2
            for q in QS:
                fp_ = psA.tile([128, LW], F32, tag="pa", name="fp_")
                for h in (0, 1):
                    nc.tensor.matmul(fp_[:, HS[h]], cs["wflbd"][:],
                                     a_t[q][:, HS[h]], start=True, stop=False)
                    nc.tensor.matmul(fp_[:, HS[h]], cs["wfp2"][:],
                                     cs["pec"][:], start=False, stop=True)
                ps_t[q] = fp_
            for q in QS:
                elu1(pA_sb, pw_tiles[q][:], ps_t[q][:], cs["bfd"], "fuse")
            # global branch: 4 (quad, half) units, stage-interleaved in pairs
            units = [(q, h) for q in QS for h in (0, 1)]
            for u0 in range(0, 4, 2):
                US = units[u0:u0 + 2]
                gp_t, ga_t = {}, {}
                for (q, h) in US:
                    pwq = pw_tiles[q]
                    psl = HS[h]
                    g1p = psG.tile([128, LW], F32, tag="pg", name="g1p")
                    nc.tensor.matmul(g1p[:, H0], cs["g1w2"][0:64, :],
                                     pwq[0:64, psl], start=True, stop=True)
                    nc.tensor.matmul(g1p[:, H1], cs["g1w2"][64:128, :],
                                     pwq[64:128, psl], start=True, stop=True)
                    gp_t[(q, h)] = g1p
                for (q, h) in US:
                    g1a = pA_sb.tile([128, LW], BF, tag="g1a", name="g1a")
                    elu1(pA_sb, g1a[:], gp_t[(q, h)][:], cs["bg1d"], "g1")
                    ga_t[(q, h)] = g1a
                for (q, h) in US:
                    g2p = psG.tile([128, LW], F32, tag="pg", name="g2p")
                    for s in (0, 1):
                        nc.tensor.matmul(g2p[:, HS[s]], cs["g2w"][:],
                                         ga_t[(q, h)][:, HS[s]],
                                         start=True, stop=True)
                    gp_t[(q, h)] = g2p
                for (q, h) in US:
                    j = 2 * q + h
                    for s in (0, 1):
                        sidx = 2 * j + s
                        nc.vector.tensor_reduce(
                            gf_all[:, sidx:sidx + 1], gp_t[(q, h)][:, HS[s]],
                            axis=mybir.AxisListType.X, op=AX.max)

          # ---- phase B (chunk): global bias + q/Q projections ----
          SPC = 4 * QPC      # samples per chunk (64)
          PPC = 2 * QPC      # pairs per chunk (32)
          ss = slice(SPC * ck, SPC * (ck + 1))
          nc.vector.tensor_scalar(gf_all[:, ss], gf_all[:, ss], cs["bg2"][:],
                                  None, op0=AX.add)
          nc.vector.tensor_copy(gf_bf[:, ss], gf_all[:, ss])
          qp_ = psG.tile([D_LOCAL, SPC], F32, tag="pg", name="qp_")
          nc.tensor.matmul(qp_[:], cs["qpwg"][:], gf_bf[:, ss], start=True,
                           stop=False)
          nc.tensor.matmul(qp_[:], cs["qpwp"][:], cprop[:, ss], start=False,
                           stop=True)
          qe = pC_sb.tile([D_LOCAL, SPC], BF, tag="qe", name="qe")
          qr = pC_sb.tile([D_LOCAL, SPC], BF, tag="qr", name="qr")
          qsb = pC_sb.tile([D_LOCAL, SPC], BF, tag="qsb", name="qsb")
          nc.scalar.activation(qe[:], qp_[:], AF.Exp, bias=cs["qpb"][:],
                               scale=1.0)
          nc.scalar.activation(qr[:], qp_[:], AF.Relu, bias=cs["qpb"][:],
                               scale=1.0)
          nc.vector.tensor_scalar(qe[:], qe[:], 1.0, -1.0, op0=AX.min,
                                  op1=AX.add)
          nc.vector.tensor_tensor(qsb[:], qe[:], qr[:], op=AX.add)
          Qp = psG.tile([128, PPC], F32, tag="pg", name="Qp")
          qs_eo = qsb[:].rearrange("p (j s) -> p s j", s=2)
          nc.tensor.matmul(Qp[0:64, :], cs["wq"][:], qs_eo[:, 0, :],
                           start=True, stop=True)
          nc.tensor.matmul(Qp[64:128, :], cs["wq"][:], qs_eo[:, 1, :],
                           start=True, stop=True, tile_position=(0, 64))
          js = slice(PPC * ck, PPC * (ck + 1))
          nc.vector.tensor_scalar(Q2[:, js], Qp[:], cs["bq2"][:], None,
                                  op0=AX.add)

          # ---- phase C: attention per pair ----
          for j in range(PPC * ck, PPC * (ck + 1)):
            pwj = pw_tiles[j // 2][:, (j % 2) * LP:(j % 2) * LP + L]
            vkp = psA.tile([128, LW], F32, tag="pa", name="vkp")
            nc.tensor.matmul(vkp[:, 0:L], cs["wvbd"][:], pwj, start=True,
                             stop=True)
            nc.tensor.matmul(vkp[:, LP:LP + L], cs["wkbd"][:], pwj, start=True,
                             stop=True)
            vks = pC_sb.tile([128, LW], BF, tag="vks", name="vks")
            nc.scalar.activation(vks[:], vkp[:], AF.Copy)
            sqbd = pC_sb.tile([128, 128], BF, tag="sqbd", name="sqbd")
            nc.vector.tensor_scalar(sqbd[:], cs["csmbd"][:], Q2[:, j:j + 1],
                                    None, op0=AX.mult)
            scp = psG.tile([128, LW], F32, tag="pg", name="scp")
            nc.tensor.matmul(scp[:, 0:L], sqbd[:], vks[:, LP:LP + L],
                             start=True, stop=True)
            esb = pC_sb.tile([128, L], BF, tag="esb", name="esb")
            sume = pC_sb.tile([128, 1], F32, tag="sume", name="sume")
            nc.scalar.activation(esb[:], scp[:, 0:L], AF.Exp,
                                 accum_out=sume[:])
            rec = pC_sb.tile([128, 1], F32, tag="rec", name="rec")
            nc.vector.reciprocal(rec[:], sume[:])
            wvt = pC_sb.tile([128, L], BF, tag="wvt", name="wvt")
            ctxu = pC_sb.tile([128, 1], F32, tag="ctxu", name="ctxu")
            nc.vector.scalar_tensor_tensor(wvt[:], esb[:], 1.0, vks[:, 0:L],
                                           op0=AX.mult, op1=AX.mult,
                                           accum_out=ctxu[:])
            nc.vector.tensor_scalar(ctx_all[:, j:j + 1], ctxu[:], rec[:],
                                    cs["cvd"][:], op0=AX.mult, op1=AX.add)

        # ================= PHASE D: output projection + stores ================
        wlp = psG.tile([128, NPAIR], F32, tag="pg", name="wlp")
        nc.tensor.matmul(wlp[:], cs["wobd"][:], ctx_all[:], start=True,
                         stop=True)
        wl = singles.tile([128, NPAIR], F32, tag="wl", name="wl")
        nc.vector.tensor_scalar(wl[:], wlp[:], cs["bod"][:], None, op0=AX.add)

        OD = D_LOCAL + D_GLOBAL  # 192
        nc.sync.dma_start(
            out=dram_ap(out_h, 0, [[1, 64], [2 * OD, NPAIR]]), in_=wl[0:64, :])
        nc.sync.dma_start(
            out=dram_ap(out_h, OD, [[1, 64], [2 * OD, NPAIR]]),
            in_=wl[64:128, :])
        nc.sync.dma_start(
            out=dram_ap(out_h, 64, [[1, D_GLOBAL], [OD, B_LOC]]), in_=gf_all[:])

    _split_multiwait(nc)
    return nc


# ----------------------------------------------------------------------------
# entry point
# ----------------------------------------------------------------------------
_CACHE = {}


def kernel(**inputs):
    shared = host_prep_shared(inputs)
    cores = host_prep_percore(inputs)

    if "nc" not in _CACHE:
        _CACHE["nc"] = build_nc(shared)
    nc = _CACHE["nc"]

    in_maps = []
    for ci in range(N_CORES):
        m = dict(cores[ci])
        for name, arr in shared.items():
            m[name] = arr
        in_maps.append(m)

    trace = bool(int(os.environ.get("AME2_TRACE", "0")))
    res = run_bass_kernel_spmd(nc, in_maps, core_ids=list(range(N_CORES)),
                               trace=trace)
    if trace and res.exec_time_ns is not None:
        _CACHE["exec_time_ns"] = res.exec_time_ns
    outs = [res.results[ci]["out"] for ci in range(N_CORES)]
    return np.concatenate(outs, axis=0).astype(np.float32)
